# revision 7
# baseline (speedup 1.0000x reference)
"""AGDN motion forecaster on 8 TRN2 NeuronCores.

Strategy: data-parallel over batch (16 items -> 2 per core). All O(B*L*L*H)
GNN message-passing work runs on-device, fully SBUF-resident (no [B,L,L,H]
HBM spill). Edge-MLP second matmul is folded into the attention/message
input blocks on host:
    eh @ Wa1_e = relu(ef@We1+be1) @ (We2@Wa1_e) + const
so per pair we need only z = relu(ef@We1+be1) and two weight-stationary
[H,H] matmuls. The outer message matmul Wm2 commutes with the softmax
weighted sum (sum_j w_ij = 1), collapsing another L^2 matmul to [L,H].
Tiny host epilogue: decoder MLP on the last timestep (16 rows).
"""

import math

import numpy as np
import ml_dtypes

B, L, D_IN, H, NLAYER, FUT, MDIM = 16, 128, 8, 128, 2, 12, 2
NCORES = 8
BPC = B // NCORES  # batch items per core
TI = 4             # query rows per chunk
CH = TI * L        # pair-columns per chunk (512)
NCH = L // TI      # chunks per sweep (32)

_CACHE = {}


def _build_bass():
    import concourse.mybir as mybir
    import concourse.tile as tile
    from concourse import bacc

    f32 = mybir.dt.float32
    bf16 = mybir.dt.bfloat16
    AF = mybir.ActivationFunctionType
    ALU = mybir.AluOpType
    AX = mybir.AxisListType

    nc = bacc.Bacc()

    eft_d = nc.dram_tensor("eft", [BPC, 4, L * L], bf16, kind="ExternalInput")
    h0t_d = nc.dram_tensor("h0t", [BPC, H, L], f32, kind="ExternalInput")
    w128_d = nc.dram_tensor("w128", [H, NLAYER * 9 * H], bf16, kind="ExternalInput")
    we1_d = nc.dram_tensor("we1", [4, NLAYER * H], bf16, kind="ExternalInput")
    wa2_d = nc.dram_tensor("wa2", [H, NLAYER], bf16, kind="ExternalInput")
    ones_d = nc.dram_tensor("onesr", [1, H], bf16, kind="ExternalInput")
    ident_d = nc.dram_tensor("ident", [H, H], f32, kind="ExternalInput")
    bias_d = nc.dram_tensor("biases", [H, NLAYER * 6 + 1], f32, kind="ExternalInput")
    grep_d = nc.dram_tensor("grep", [H, NLAYER * H], f32, kind="ExternalInput")
    brep_d = nc.dram_tensor("brep", [H, NLAYER * H], f32, kind="ExternalInput")
    hout_d = nc.dram_tensor("hout", [BPC, L, H], f32, kind="ExternalOutput")

    # w128 slot order
    IWEA, IWEM, IWM2, IWO1S, IWO1A, IWO2, IWA1S, IWA1D, IWM1S = range(9)
    # bias column order
    CBE1, CBEA, CBEM, CBM2, CBO1, CBO2 = range(6)

    with tile.TileContext(nc) as tc:
        with (
            tc.tile_pool(name="consts", bufs=1) as consts,
            tc.tile_pool(name="state", bufs=2) as state,
            tc.tile_pool(name="work", bufs=3) as work,
            tc.tile_pool(name="big", bufs=1) as bigpool,
            tc.tile_pool(name="dramp", bufs=2, space="DRAM") as dram,
        ):
            w128_sb = consts.tile([H, NLAYER * 9 * H], bf16)
            nc.sync.dma_start(w128_sb, w128_d[:, :], single_packet=True)

            def W(lyr, idx):
                o = (lyr * 9 + idx) * H
                return w128_sb[:, o:o + H]

            we1_sb = consts.tile([4, NLAYER * H], bf16)
            nc.sync.dma_start(we1_sb, we1_d[:, :], single_packet=True)
            wa2_sb = consts.tile([H, NLAYER], bf16)
            nc.sync.dma_start(wa2_sb, wa2_d[:, :], single_packet=True)
            ones_sb = consts.tile([1, H], bf16)
            nc.sync.dma_start(ones_sb, ones_d[:, :], single_packet=True)
            ident_sb = consts.tile([H, H], f32)
            nc.sync.dma_start(ident_sb, ident_d[:, :], single_packet=True)
            bias_sb = consts.tile([H, NLAYER * 6 + 1], f32)
            nc.sync.dma_start(bias_sb, bias_d[:, :], single_packet=True)

            def BIAS(lyr, c):
                o = lyr * 6 + c
                return bias_sb[:, o:o + 1]

            grep_sb = consts.tile([H, NLAYER * H], f32)
            nc.sync.dma_start(grep_sb, grep_d[:, :], single_packet=True)
            brep_sb = consts.tile([H, NLAYER * H], f32)
            nc.sync.dma_start(brep_sb, brep_d[:, :], single_packet=True)

            for it in range(BPC):
                ef_sb = bigpool.tile([4, L * L], bf16, tag="ef")
                nc.sync.dma_start(ef_sb, eft_d[it], single_packet=True)
                hT = state.tile([H, L], f32, tag="hT")
                nc.sync.dma_start(hT, h0t_d[it], single_packet=True)
                hT_bf = state.tile([H, L], bf16, tag="hTbf")
                nc.scalar.activation(hT_bf, hT, AF.Copy)

                for lyr in range(NLAYER):
                    # per-layer projections of current node state
                    sda = []
                    with tc.tile_pool(name=f"pro{it}{lyr}", bufs=3, space="PSUM") as pp:
                        for idx in (IWA1S, IWA1D, IWM1S):
                            ps = pp.tile([H, L], f32, tag="pro")
                            nc.tensor.matmul(ps, W(lyr, idx), hT_bf, start=True, stop=True)
                            sb = state.tile([H, L], f32, tag=f"sda{idx}")
                            nc.scalar.activation(sb, ps, AF.Copy)
                            sda.append(sb)
                    sa_sb, da_sb, msrc_sb = sda

                    msgrelu = bigpool.tile([H, L * L], bf16, tag="msgrelu")
                    att_dram = dram.tile([L, L], f32, tag="attd")

                    # phase 1: per pair z, attention logits, message pre-act
                    with tc.tile_pool(name=f"p1_{it}{lyr}", bufs=2, space="PSUM") as pp:
                        for c in range(NCH):
                            sl = slice(c * CH, (c + 1) * CH)
                            zp = pp.tile([H, CH], f32, tag="zp")
                            nc.tensor.matmul(zp, we1_sb[:, lyr * H:(lyr + 1) * H],
                                             ef_sb[:, sl], start=True, stop=True)
                            z_bf = work.tile([H, CH], bf16, tag="z")
                            nc.scalar.activation(z_bf, zp, AF.Relu, bias=BIAS(lyr, CBE1))
                            am = pp.tile([H, 2 * CH], f32, tag="am")
                            nc.tensor.matmul(am[:, :CH], W(lyr, IWEA), z_bf, start=True, stop=True)
                            nc.tensor.matmul(am[:, CH:], W(lyr, IWEM), z_bf, start=True, stop=True)

                            v = work.tile([H, TI, L], f32, tag="v")
                            sa_bc = sa_sb[:, c * TI:(c + 1) * TI].unsqueeze(2).broadcast_to([H, TI, L])
                            da_bc = da_sb.unsqueeze(1).broadcast_to([H, TI, L])
                            a_view = am[:, :CH].rearrange("p (s j) -> p s j", j=L)
                            nc.vector.tensor_add(v, a_view, sa_bc)
                            nc.vector.tensor_add(v, v, da_bc)
                            tanh_bf = work.tile([H, CH], bf16, tag="tanh")
                            nc.scalar.activation(tanh_bf, v.rearrange("p s j -> p (s j)"),
                                                 AF.Tanh, bias=BIAS(lyr, CBEA))
                            attp = pp.tile([1, CH], f32, tag="attp")
                            nc.tensor.matmul(attp, wa2_sb[:, lyr:lyr + 1], tanh_bf,
                                             start=True, stop=True)
                            att_st = work.tile([1, CH], f32, tag="attst")
                            nc.scalar.activation(att_st, attp, AF.Copy)
                            nc.sync.dma_start(
                                att_dram.rearrange("a b -> (a b)").unsqueeze(0)[:, sl], att_st)

                            mpre = work.tile([H, TI, L], f32, tag="mpre")
                            msrc_bc = msrc_sb[:, c * TI:(c + 1) * TI].unsqueeze(2).broadcast_to([H, TI, L])
                            m_view = am[:, CH:].rearrange("p (s j) -> p s j", j=L)
                            nc.vector.tensor_add(mpre, m_view, msrc_bc)
                            nc.scalar.activation(msgrelu[:, sl],
                                                 mpre.rearrange("p s j -> p (s j)"),
                                                 AF.Relu, bias=BIAS(lyr, CBEM))

                    # softmax over keys (rows = queries after DRAM-bounce reshape)
                    att_sq = state.tile([L, L], f32, tag="attsq")
                    nc.sync.dma_start(att_sq, att_dram, single_packet=True)
                    rowmax = state.tile([L, 1], f32, tag="rowmax")
                    nc.vector.reduce_max(rowmax, att_sq, axis=AX.X)
                    negmax = state.tile([L, 1], f32, tag="negmax")
                    nc.vector.tensor_scalar_mul(negmax, rowmax, -1.0)
                    esq = state.tile([L, L], f32, tag="esq")
                    nc.scalar.activation(esq, att_sq, AF.Exp, bias=negmax[:, 0:1])
                    ssum = state.tile([L, 1], f32, tag="ssum")
                    nc.vector.reduce_sum(ssum, esq, axis=AX.X)
                    rinv = state.tile([L, 1], f32, tag="rinv")
                    nc.vector.reciprocal(rinv, ssum)
                    w_bf = state.tile([L, L], bf16, tag="wbf")
                    nc.vector.tensor_scalar_mul(w_bf, esq, rinv[:, 0:1])
                    w_dram = dram.tile([L, L], bf16, tag="wd")
                    nc.sync.dma_start(w_dram, w_bf)
                    w_flat = bigpool.tile([1, L * L], bf16, tag="wflat")
                    nc.sync.dma_start(w_flat, w_dram.rearrange("a b -> (a b)").unsqueeze(0), single_packet=True)

                    # phase 2: weighted message aggregation
                    aggpre = state.tile([H, L], f32, tag="aggpre")
                    with tc.tile_pool(name=f"p2_{it}{lyr}", bufs=2, space="PSUM") as pp:
                        for c in range(NCH):
                            sl = slice(c * CH, (c + 1) * CH)
                            wr = pp.tile([H, CH], f32, tag="wr")
                            nc.tensor.matmul(wr, ones_sb, w_flat[:, sl], start=True, stop=True)
                            wr_bf = work.tile([H, CH], bf16, tag="wrbf")
                            nc.scalar.activation(wr_bf, wr, AF.Copy)
                            prod = work.tile([H, TI, L], f32, tag="prod")
                            nc.vector.tensor_mul(
                                prod,
                                msgrelu[:, sl].rearrange("p (s j) -> p s j", j=L),
                                wr_bf.rearrange("p (s j) -> p s j", j=L))
                            nc.vector.reduce_sum(aggpre[:, c * TI:(c + 1) * TI], prod, axis=AX.X)

                    aggpre_bf = state.tile([H, L], bf16, tag="aggprebf")
                    nc.scalar.activation(aggpre_bf, aggpre, AF.Copy)

                    # epilogue: out-MLP, residual, LayerNorm
                    with tc.tile_pool(name=f"ep_{it}{lyr}", bufs=1, space="PSUM") as pp:
                        aggT = pp.tile([H, L], f32, tag="aggT")
                        nc.tensor.matmul(aggT, W(lyr, IWM2), aggpre_bf, start=True, stop=True)
                        agg_bf = state.tile([H, L], bf16, tag="aggbf")
                        nc.scalar.activation(agg_bf, aggT, AF.Identity, bias=BIAS(lyr, CBM2))
                        o1 = pp.tile([H, L], f32, tag="o1")
                        nc.tensor.matmul(o1, W(lyr, IWO1S), hT_bf, start=True, stop=False)
                        nc.tensor.matmul(o1, W(lyr, IWO1A), agg_bf, start=False, stop=True)
                        r1_bf = state.tile([H, L], bf16, tag="r1bf")
                        nc.scalar.activation(r1_bf, o1, AF.Relu, bias=BIAS(lyr, CBO1))
                        o2 = pp.tile([H, L], f32, tag="o2")
                        nc.tensor.matmul(o2, W(lyr, IWO2), r1_bf, start=True, stop=True)
                        resid = state.tile([H, L], f32, tag="resid")
                        nc.vector.scalar_tensor_tensor(resid, o2, BIAS(lyr, CBO2), hT,
                                                       op0=ALU.add, op1=ALU.add)
                        rT = pp.tile([L, H], f32, tag="rT")
                        nc.tensor.transpose(rT, resid, ident_sb)
                        musum = state.tile([L, 1], f32, tag="musum")
                        nc.vector.reduce_sum(musum, rT, axis=AX.X)
                        mu = state.tile([L, 1], f32, tag="mu")
                        nc.vector.tensor_scalar_mul(mu, musum, 1.0 / H)
                        cen = state.tile([L, H], f32, tag="cen")
                        nc.vector.tensor_scalar_sub(cen, rT, mu[:, 0:1])
                        sqd = state.tile([L, H], f32, tag="sqd")
                        varsum = state.tile([L, 1], f32, tag="varsum")
                        nc.scalar.activation(sqd, cen, AF.Square, accum_out=varsum)
                        sd = state.tile([L, 1], f32, tag="sd")
                        nc.scalar.activation(sd, varsum, AF.Sqrt, scale=1.0 / H,
                                             bias=bias_sb[:, NLAYER * 6:NLAYER * 6 + 1])
                        rstd = state.tile([L, 1], f32, tag="rstd")
                        nc.vector.reciprocal(rstd, sd)
                        hnew = state.tile([L, H], f32, tag="hnew")
                        nc.vector.scalar_tensor_tensor(hnew, cen, rstd[:, 0:1],
                                                       grep_sb[:, lyr * H:(lyr + 1) * H],
                                                       op0=ALU.mult, op1=ALU.mult)
                        hnewb = state.tile([L, H], f32, tag="hnewb")
                        nc.vector.tensor_add(hnewb, hnew, brep_sb[:, lyr * H:(lyr + 1) * H])
                        if lyr == NLAYER - 1:
                            nc.sync.dma_start(hout_d[it], hnewb)
                        else:
                            hT2 = pp.tile([H, L], f32, tag="hT2")
                            nc.tensor.transpose(hT2, hnewb, ident_sb)
                            hT = state.tile([H, L], f32, tag="hT")
                            nc.scalar.activation(hT, hT2, AF.Copy)
                            hT_bf = state.tile([H, L], bf16, tag="hTbf")
                            nc.scalar.activation(hT_bf, hT2, AF.Copy)
    nc.finalize()
    return nc


def _prep(inputs):
    """Host-side: edge features, folded weights, per-core in_maps."""
    f = {k: np.asarray(v, dtype=np.float32) for k, v in inputs.items()}
    bf = ml_dtypes.bfloat16
    inv = 1.0 / math.sqrt(H)

    hist = f["history"]
    lat, lon, src = hist[..., 0], hist[..., 1], hist[..., 6]
    ts = np.cumsum(np.maximum(hist[..., 5], 0.0), axis=1)
    dlat = lat[:, :, None] - lat[:, None, :]
    dlon = lon[:, :, None] - lon[:, None, :]
    dist = np.sqrt(dlat * dlat + dlon * dlon + 1e-8)
    dt = np.abs(ts[:, :, None] - ts[:, None, :]) / 300.0
    same_src = (src[:, :, None] == src[:, None, :]).astype(np.float32)
    eye = np.broadcast_to(np.eye(L, dtype=np.float32), (B, L, L))
    ef = np.stack([dist, dt, same_src, eye], axis=-1)          # [B,L,L,4]
    eft = ef.transpose(0, 3, 1, 2).reshape(B, 4, L * L)        # [B,4,L*L]

    h0 = hist @ f["Wp"] + f["bp"]                              # [B,L,H]
    h0t = np.ascontiguousarray(h0.transpose(0, 2, 1))          # [B,H,L]

    w128 = np.zeros((NLAYER, 9, H, H), np.float32)
    biases = np.zeros((NLAYER, H, 6), np.float32)
    we1 = np.zeros((NLAYER, 4, H), np.float32)
    wa2 = np.zeros((NLAYER, H, 1), np.float32)
    grep = np.zeros((NLAYER, H, H), np.float32)
    brep = np.zeros((NLAYER, H, H), np.float32)
    for lyr in range(NLAYER):
        Wa1, Wm1 = f["Wa1"][lyr], f["Wm1"][lyr]
        We2, be2 = f["We2"][lyr], f["be2"][lyr]
        w128[lyr, 0] = We2 @ Wa1[2 * H:]                       # Wea
        w128[lyr, 1] = We2 @ Wm1[H:]                           # Wem
        w128[lyr, 2] = f["Wm2"][lyr]
        w128[lyr, 3] = f["Wo1"][lyr][:H]
        w128[lyr, 4] = f["Wo1"][lyr][H:]
        w128[lyr, 5] = f["Wo2"][lyr]
        w128[lyr, 6] = Wa1[:H]
        w128[lyr, 7] = Wa1[H:2 * H]
        w128[lyr, 8] = Wm1[:H]
        biases[lyr, :, 0] = f["be1"][lyr]
        biases[lyr, :, 1] = be2 @ Wa1[2 * H:] + f["ba1"][lyr]  # bea
        biases[lyr, :, 2] = be2 @ Wm1[H:] + f["bm1"][lyr]      # bem
        biases[lyr, :, 3] = f["bm2"][lyr]
        biases[lyr, :, 4] = f["bo1"][lyr]
        biases[lyr, :, 5] = f["bo2"][lyr]
        we1[lyr] = f["We1"][lyr]
        wa2[lyr, :, 0] = f["wa2"][lyr] * inv                   # fold 1/sqrt(H)
        grep[lyr] = np.tile(f["ln_g"][lyr][None, :], (H, 1))
        brep[lyr] = np.tile(f["ln_b"][lyr][None, :], (H, 1))

    shared = {
        "w128": np.ascontiguousarray(w128.transpose(2, 0, 1, 3).reshape(H, -1)).astype(bf),
        "we1": np.ascontiguousarray(we1.transpose(1, 0, 2).reshape(4, -1)).astype(bf),
        "wa2": np.ascontiguousarray(wa2[:, :, 0].T).astype(bf),
        "onesr": np.ones((1, H), bf),
        "ident": np.eye(H, dtype=np.float32),
        "biases": np.concatenate([biases.transpose(1, 0, 2).reshape(H, -1),
                                  np.full((H, 1), 1e-5, np.float32)], axis=1),
        "grep": np.ascontiguousarray(grep.transpose(1, 0, 2).reshape(H, -1)),
        "brep": np.ascontiguousarray(brep.transpose(1, 0, 2).reshape(H, -1)),
    }
    in_maps = []
    for c in range(NCORES):
        s = slice(c * BPC, (c + 1) * BPC)
        m = dict(shared)
        m["eft"] = eft[s].astype(bf)
        m["h0t"] = h0t[s]
        in_maps.append(m)
    return f, in_maps


def _decode(f, h):
    """Host epilogue: decoder MLP on last valid timestep."""
    hist = f["history"]
    vc = np.clip(np.sum(f["history_mask"], axis=1).astype(np.int32), 1, L)
    last = vc - 1
    bidx = np.arange(B)
    ds = np.concatenate([hist[bidx, last], h[bidx, last]], axis=-1)
    mu = ds.mean(-1, keepdims=True)
    var = ((ds - mu) ** 2).mean(-1, keepdims=True)
    x = (ds - mu) / np.sqrt(var + 1e-5) * f["hg"] + f["hb"]
    pred = np.maximum(x @ f["Wh1"] + f["bh1"], 0.0) @ f["Wh2"] + f["bh2"]
    pred = pred.reshape(B, FUT, MDIM)
    pred = np.nan_to_num(pred, nan=0.0, posinf=1e4, neginf=-1e4)
    return pred.astype(np.float32)


def kernel(**inputs):
    from concourse.bass_utils import run_bass_kernel_spmd

    if "nc" not in _CACHE:
        _CACHE["nc"] = _build_bass()
    nc = _CACHE["nc"]

    f, in_maps = _prep(inputs)
    res = run_bass_kernel_spmd(nc, in_maps, core_ids=list(range(NCORES)))
    h = np.concatenate([np.asarray(r["hout"], np.float32) for r in res.results], axis=0)

    # mask multiply (mask is all-ones in this problem's input spec)
    vm = (np.asarray(inputs["history_mask"], np.float32) > 0).astype(np.float32)[..., None]
    h = h * vm
    pred = _decode(f, h)
    return pred, h
